# revision 7
# baseline (speedup 1.0000x reference)
"""CRF Viterbi decode (tf.contrib.crf.crf_decode + one_hot) on 8 TRN2 cores.

Data-parallel over batch: each of the 8 NeuronCores processes 128 of the
1024 sequences (batch rows on SBUF partitions). Per core the whole DP is
SBUF-resident and no per-step backpointers are materialized:

  forward (t = 1..511) keeps the full score history shist[:, t, :]:
    x[b,cc,cp] = A_T[cc,cp] + s_{t-1}[b,cp]     tensor_tensor add (s broadcast)
    raw[b,cc]  = max_cp x                       tensor_reduce axis=X
    s_t        = raw + pot[:,t]                 tensor_tensor add
    s_t        = s_{t-1} where t >= len[b]      copy_predicated

  backtrace recomputes only the traced argmax per step:
    ohT  = onehot(tag)^T (bf16)                 PE transpose
    asel = A[:, tag_b] per batch row            PE matmul (one-hot matvec, f32)
    xcol = s_t + asel                           tensor_tensor (PSUM src)
    tag  = first-argmax(xcol) if t+1 < len      Max8 + MaxIndex + copy_predicated
    out[:, t, :] = onehot(tag)                  tensor_scalar is_equal (+cast)

Score arithmetic replicates the reference's fp32 ops exactly (same adds,
exact max via one-hot matvec with exact 0/1 weights, first-index argmax),
so the output matches bit-for-bit.
"""
from contextlib import ExitStack

import numpy as np

B, T, C = 1024, 512, 48
NCORES = 8
P = B // NCORES  # 128 batch rows per core
CHUNK = 64

_CACHE = {}


def _register_seg_maxplus():
    """Custom DVE op: one streaming pass computing, per partition, the
    SEGMENTED running max of (in0 + in1) with segment length 48 (SUB_DIM
    boundaries of the 3D in0 AP). The segment tails are the grouped maxes.

    The stock Spec DSL has no segmented-scan reset, but the generated FSM
    already supports a SUB_DIM_DONE-triggered step state with per-stage
    overrides (production-tested by TENSOR_PAGED_MASK). We patch
    `_scan_overrides` to emit, for scans tagged `_ant_reset`, a step
    override that re-seeds the scan register from the current element:
    state := expr  (instead of state := max(state, expr)).
    """
    import concourse.dve_spec as ds
    from concourse.dve_ops import (OPS, CUSTOM_DVE_SPECS, DveOp,
                                   _SUB_OPCODE_FOR_NAME)
    from concourse.dve_uop import DveOpSpec

    name = "SEG_MAXPLUS_ANT"
    for o in OPS:
        if o.name == name:
            return o

    if not getattr(ds, "_ant_seg_patch", False):
        _orig_overrides = ds._scan_overrides

        def _patched(scans, node_stage):
            seed, step = _orig_overrides(scans, node_stage)
            for sc in scans:
                if getattr(sc, "_ant_reset", False):
                    step[node_stage[sc]] = ds._Stage(ds.AluOp.BYPASS, sc.expr)
            return seed, step

        ds._scan_overrides = _patched
        ds._ant_seg_patch = True

    body = ds.scan(ds.AluOp.MAX, ds.Src0 + ds.Src1)
    object.__setattr__(body, "_ant_reset", True)

    def _ref(in0, in1, c0, c1, c2):
        x = np.asarray(in0, np.float32) + np.asarray(in1, np.float32).reshape(
            in0.shape)
        p = x.shape[0]
        x3 = x.reshape(p, -1, x.shape[-1])
        return np.maximum.accumulate(x3, axis=-1).reshape(in0.shape)

    spec = ds.Spec(body=body, reference=_ref)
    row = 1 + len(OPS)
    _SUB_OPCODE_FOR_NAME[name] = row
    shas = {}
    for ver in ("v3", "v4"):
        try:
            shas[ver] = DveOpSpec(name=name, opcode=row,
                                  uops=ds.lower(spec, ver=ver),
                                  rd1_en=True).sha(ver)
        except Exception:
            pass
    op = DveOp(name, spec, subdim=True, uops_sha=shas)
    OPS.append(op)
    CUSTOM_DVE_SPECS[name] = spec
    return op


def crf_body(tc, outs, ins, T=T, CHUNK=CHUNK):
    import concourse.tile as tile  # noqa: F401
    from concourse import mybir
    from concourse.masks import make_identity

    F32 = mybir.dt.float32
    BF16 = mybir.dt.bfloat16
    U8 = mybir.dt.uint8
    I32 = mybir.dt.int32
    U32 = mybir.dt.uint32
    ALU = mybir.AluOpType

    nc = tc.nc
    segop = _register_seg_maxplus()
    pot = ins["pot"]      # [P, T, C] f32 dram
    lens = ins["lens"]    # [P, 1] f32 dram
    at = ins["at"]        # [P, C, C] f32 dram  (at[b,cc,cp] = A[cp,cc])
    outd = outs["out"]    # [P, T, C] f32 dram
    assert T % CHUNK == 0

    with ExitStack() as ctx:
        singles = ctx.enter_context(tc.tile_pool(name="singles", bufs=1))
        potp = ctx.enter_context(tc.tile_pool(name="potp", bufs=2))
        xp = ctx.enter_context(tc.tile_pool(name="xp", bufs=2))
        smal = ctx.enter_context(tc.tile_pool(name="smal", bufs=4))
        outp = ctx.enter_context(tc.tile_pool(name="outp", bufs=2))
        psp = ctx.enter_context(tc.tile_pool(name="psp", bufs=2, space="PSUM"))

        at_sb = singles.tile([P, C, C], F32)
        nc.sync.dma_start(out=at_sb, in_=at)
        at_pe = singles.tile([C, C], F32)  # A_T rows on partitions (PE rhs)
        nc.sync.dma_start(out=at_pe, in_=at[0, :, :])
        lens_sb = singles.tile([P, 1], F32)
        nc.sync.dma_start(out=lens_sb, in_=lens)
        ident = singles.tile([P, P], BF16)
        make_identity(nc, ident[:, :])

        iota_t = singles.tile([P, T], I32)
        nc.gpsimd.iota(iota_t, pattern=[[1, T]], base=0, channel_multiplier=0)
        iota_c = singles.tile([P, C], I32)
        nc.gpsimd.iota(iota_c, pattern=[[1, C]], base=0, channel_multiplier=0)
        iota_c_f = singles.tile([P, C], F32)
        nc.vector.tensor_copy(out=iota_c_f, in_=iota_c)

        # m[b,t] = t < len[b]; minv = !m. Integer masks for CopyPredicated
        # (and only plain tensor_copy may write u8 on this HW).
        mf = singles.tile([P, T], F32)
        nc.vector.tensor_scalar(out=mf, in0=iota_t, scalar1=lens_sb[:, :],
                                scalar2=None, op0=ALU.is_lt)
        minvf = singles.tile([P, T], F32)
        nc.vector.tensor_scalar(out=minvf, in0=mf, scalar1=-1.0, scalar2=1.0,
                                op0=ALU.mult, op1=ALU.add)
        m = singles.tile([P, T], U8)
        nc.vector.tensor_copy(out=m, in_=mf)
        minv = singles.tile([P, T], U8)
        nc.vector.tensor_copy(out=minv, in_=minvf)

        # score history: shist[:, t, :] = s_t
        shist = singles.tile([P, T, C], F32)
        nc.sync.dma_start(out=shist[:, 0, :], in_=pot[:, 0, :])

        # --- forward ---
        pot_sb = None
        for t in range(1, T):
            if t % CHUNK == 0 or pot_sb is None:
                c0 = (t // CHUNK) * CHUNK
                pot_sb = potp.tile([P, CHUNK, C], F32, tag="pot")
                nc.sync.dma_start(out=pot_sb, in_=pot[:, c0:c0 + CHUNK, :])
            col = t % CHUNK

            x = xp.tile([P, C, C], F32, tag="x")
            nc.vector._custom_dve(
                segop, out=x, in0=at_sb[:, :, :],
                in1=shist[:, t - 1, :].unsqueeze(1).broadcast_to([P, C, C]))
            nc.vector.tensor_tensor(shist[:, t, :], x[:, :, C - 1],
                                    pot_sb[:, col, :], ALU.add)
            nc.vector.copy_predicated(
                out=shist[:, t, :],
                mask=minv[:, t:t + 1].broadcast_to([P, C]),
                data=shist[:, t - 1, :])

        # --- last tag ---
        v8 = smal.tile([P, 8], F32, tag="v8")
        nc.vector.max(out=v8, in_=shist[:, T - 1, :])
        i8 = smal.tile([P, 8], U32, tag="i8")
        nc.vector.max_index(out=i8, in_max=v8, in_values=shist[:, T - 1, :])
        tag = singles.tile([P, 1], F32)
        nc.vector.tensor_copy(out=tag, in_=i8[:, 0:1])

        # --- backtrace ---
        # Two independent half-batch chains (partitions 0-63 / 64-127),
        # interleaved so the PE/ACT/DVE stages of the two serial chains
        # overlap. bf16 one-hot state: exact for 0/1, and the PE transpose
        # of bf16 is a single matmul pass (fp32 needs two).
        H = P // 2
        sls = [slice(0, H), slice(H, P)]
        ohs = []
        out_sb = outp.tile([P, CHUNK, C], F32, tag="oc")
        for h, sl in enumerate(sls):
            oh = smal.tile([P, C], BF16, tag=f"oh{h}")
            nc.vector.tensor_scalar(out=oh[sl, :], in0=iota_c_f[sl, :],
                                    scalar1=tag[sl, :], scalar2=None,
                                    op0=ALU.is_equal)
            nc.scalar.copy(out=out_sb[sl, CHUNK - 1, :], in_=oh[sl, :])
            ohs.append(oh)
        for t in range(T - 2, -1, -1):
            ci = t // CHUNK
            col = t % CHUNK
            if col == CHUNK - 1:
                new_sb = outp.tile([P, CHUNK, C], F32, tag="oc")
                out_sb_hi, out_sb = out_sb, new_sb
            for h, sl in enumerate(sls):
                oh = ohs[h]
                ps_ohT = psp.tile([C, H], BF16, tag=f"ohT{h}")
                nc.tensor.transpose(ps_ohT, oh[sl, :],
                                    ident[sl, h * H:(h + 1) * H])
                sb_ohT = smal.tile([C, H], F32, tag=f"sbohT{h}")
                nc.scalar.copy(out=sb_ohT, in_=ps_ohT)
                ps_asel = psp.tile([P, C], F32, tag=f"asel{h}")
                nc.tensor.matmul(ps_asel[sl, :], sb_ohT, at_pe,
                                 start=True, stop=True)
                xcol = smal.tile([P, C], F32, tag=f"xcol{h}")
                nc.vector.tensor_tensor(xcol[sl, :], shist[sl, t, :],
                                        ps_asel[sl, :], ALU.add)
                bv8 = smal.tile([P, 8], F32, tag=f"bv8{h}")
                nc.vector.max(out=bv8[sl, :], in_=xcol[sl, :])
                bi8 = smal.tile([P, 8], U32, tag=f"bi8{h}")
                nc.vector.max_index(out=bi8[sl, :], in_max=bv8[sl, :],
                                    in_values=xcol[sl, :])
                tag_new = smal.tile([P, 1], F32, tag=f"tagn{h}")
                nc.vector.tensor_copy(out=tag_new[sl, :], in_=bi8[sl, 0:1])
                nc.vector.copy_predicated(out=tag[sl, :],
                                          mask=m[sl, t + 1:t + 2],
                                          data=tag_new[sl, :])
                oh = smal.tile([P, C], BF16, tag=f"oh{h}")
                nc.vector.tensor_scalar(out=oh[sl, :], in0=iota_c_f[sl, :],
                                        scalar1=tag[sl, :], scalar2=None,
                                        op0=ALU.is_equal)
                ohs[h] = oh
                # output column (f32) off the chain, on the Scalar engine
                nc.scalar.copy(out=out_sb[sl, col, :], in_=oh[sl, :])
            if col == CHUNK - 1:
                hi0 = (ci + 1) * CHUNK
                nc.sync.dma_start(out=outd[:, hi0:hi0 + CHUNK, :],
                                  in_=out_sb_hi)
        nc.sync.dma_start(out=outd[:, 0:CHUNK, :], in_=out_sb)


def _build_module():
    import concourse.bacc as bacc
    import concourse.tile as tile
    from concourse import mybir

    F32 = mybir.dt.float32
    nc = bacc.Bacc("TRN2", debug=False, enable_asserts=False,
                   target_bir_lowering=False, num_devices=NCORES)
    ins = {
        "pot": nc.dram_tensor("pot", [P, T, C], F32, kind="ExternalInput").ap(),
        "lens": nc.dram_tensor("lens", [P, 1], F32, kind="ExternalInput").ap(),
        "at": nc.dram_tensor("at", [P, C, C], F32, kind="ExternalInput").ap(),
    }
    outs = {
        "out": nc.dram_tensor("out", [P, T, C], F32, kind="ExternalOutput").ap(),
    }
    with tile.TileContext(nc) as tc:
        crf_body(tc, outs, ins)
    nc.compile()
    return nc


def _get_module():
    if "nc" not in _CACHE:
        _CACHE["nc"] = _build_module()
    return _CACHE["nc"]


def _run(inputs, **spmd_kwargs):
    from concourse.bass_utils import run_bass_kernel_spmd

    potentials = np.ascontiguousarray(inputs["potentials"], dtype=np.float32)
    seq_lens = np.asarray(inputs["sequence_lengths"])
    transitions = np.ascontiguousarray(inputs["transitions"], dtype=np.float32)
    assert potentials.shape == (B, T, C)

    at_host = np.broadcast_to(
        np.ascontiguousarray(transitions.T)[None], (P, C, C))
    at_host = np.ascontiguousarray(at_host)
    lens_f = seq_lens.reshape(B, 1).astype(np.float32)

    in_maps = []
    for c in range(NCORES):
        sl = slice(c * P, (c + 1) * P)
        in_maps.append({
            "pot": np.ascontiguousarray(potentials[sl]),
            "lens": np.ascontiguousarray(lens_f[sl]),
            "at": at_host,
        })

    nc = _get_module()
    res = run_bass_kernel_spmd(nc, in_maps, core_ids=list(range(NCORES)),
                               **spmd_kwargs)
    out = np.concatenate([r["out"] for r in res.results], axis=0)
    return out.astype(np.float32), res


def kernel(**inputs) -> np.ndarray:
    out, _ = _run(inputs)
    return out


# revision 8
# speedup vs baseline: 1.1871x; 1.1871x over previous
"""CRF Viterbi decode (tf.contrib.crf.crf_decode + one_hot) on 8 TRN2 cores.

Data-parallel over batch: each of the 8 NeuronCores processes 128 of the
1024 sequences (batch rows on SBUF partitions). Per core the whole DP is
SBUF-resident and no per-step backpointers are materialized:

  forward (t = 1..511) keeps the full score history shist[:, t, :]:
    x[b,cc,cp] = A_T[cc,cp] + s_{t-1}[b,cp]     tensor_tensor add (s broadcast)
    raw[b,cc]  = max_cp x                       tensor_reduce axis=X
    s_t        = raw + pot[:,t]                 tensor_tensor add
    s_t        = s_{t-1} where t >= len[b]      copy_predicated

  backtrace recomputes only the traced argmax per step:
    ohT  = onehot(tag)^T (bf16)                 PE transpose
    asel = A[:, tag_b] per batch row            PE matmul (one-hot matvec, f32)
    xcol = s_t + asel                           tensor_tensor (PSUM src)
    tag  = first-argmax(xcol) if t+1 < len      Max8 + MaxIndex + copy_predicated
    out[:, t, :] = onehot(tag)                  tensor_scalar is_equal (+cast)

Score arithmetic replicates the reference's fp32 ops exactly (same adds,
exact max via one-hot matvec with exact 0/1 weights, first-index argmax),
so the output matches bit-for-bit.
"""
from contextlib import ExitStack

import numpy as np

B, T, C = 1024, 512, 48
NCORES = 8
P = B // NCORES  # 128 batch rows per core
CHUNK = 64

_CACHE = {}


def _register_seg_maxplus():
    """Custom DVE op: one streaming pass computing, per partition, the
    SEGMENTED running max of (in0 + in1) with segment length 48 (SUB_DIM
    boundaries of the 3D in0 AP). The segment tails are the grouped maxes.

    The stock Spec DSL has no segmented-scan reset, but the generated FSM
    already supports a SUB_DIM_DONE-triggered step state with per-stage
    overrides (production-tested by TENSOR_PAGED_MASK). We patch
    `_scan_overrides` to emit, for scans tagged `_ant_reset`, a step
    override that re-seeds the scan register from the current element:
    state := expr  (instead of state := max(state, expr)).
    """
    import concourse.dve_spec as ds
    from concourse.dve_ops import (OPS, CUSTOM_DVE_SPECS, DveOp,
                                   _SUB_OPCODE_FOR_NAME)
    from concourse.dve_uop import DveOpSpec

    name = "SEG_MAXPLUS_ANT"
    for o in OPS:
        if o.name == name:
            return o

    if not getattr(ds, "_ant_seg_patch", False):
        _orig_overrides = ds._scan_overrides

        def _patched(scans, node_stage):
            seed, step = _orig_overrides(scans, node_stage)
            for sc in scans:
                if getattr(sc, "_ant_reset", False):
                    step[node_stage[sc]] = ds._Stage(ds.AluOp.BYPASS, sc.expr)
            return seed, step

        ds._scan_overrides = _patched
        ds._ant_seg_patch = True

    body = ds.scan(ds.AluOp.MAX, ds.Src0 + ds.Src1)
    object.__setattr__(body, "_ant_reset", True)

    def _ref(in0, in1, c0, c1, c2):
        x = np.asarray(in0, np.float32) + np.asarray(in1, np.float32).reshape(
            in0.shape)
        p = x.shape[0]
        x3 = x.reshape(p, -1, x.shape[-1])
        return np.maximum.accumulate(x3, axis=-1).reshape(in0.shape)

    spec = ds.Spec(body=body, reference=_ref)
    row = 1 + len(OPS)
    _SUB_OPCODE_FOR_NAME[name] = row
    shas = {}
    for ver in ("v3", "v4"):
        try:
            shas[ver] = DveOpSpec(name=name, opcode=row,
                                  uops=ds.lower(spec, ver=ver),
                                  rd1_en=True).sha(ver)
        except Exception:
            pass
    op = DveOp(name, spec, subdim=True, uops_sha=shas)
    OPS.append(op)
    CUSTOM_DVE_SPECS[name] = spec
    return op


def crf_body(tc, outs, ins, T=T, CHUNK=CHUNK):
    import concourse.tile as tile  # noqa: F401
    from concourse import mybir
    from concourse.masks import make_identity

    F32 = mybir.dt.float32
    BF16 = mybir.dt.bfloat16
    U8 = mybir.dt.uint8
    I32 = mybir.dt.int32
    U32 = mybir.dt.uint32
    ALU = mybir.AluOpType

    nc = tc.nc
    segop = _register_seg_maxplus()
    pot = ins["pot"]      # [P, T, C] f32 dram
    lens = ins["lens"]    # [P, 1] f32 dram
    at = ins["at"]        # [P, C, C] f32 dram  (at[b,cc,cp] = A[cp,cc])
    outd = outs["out"]    # [P, T, C] f32 dram
    assert T % CHUNK == 0

    with ExitStack() as ctx:
        singles = ctx.enter_context(tc.tile_pool(name="singles", bufs=1))
        potp = ctx.enter_context(tc.tile_pool(name="potp", bufs=2))
        xp = ctx.enter_context(tc.tile_pool(name="xp", bufs=2))
        smal = ctx.enter_context(tc.tile_pool(name="smal", bufs=4))
        outp = ctx.enter_context(tc.tile_pool(name="outp", bufs=2))
        psp = ctx.enter_context(tc.tile_pool(name="psp", bufs=2, space="PSUM"))

        at_sb = singles.tile([P, C, C], F32)
        nc.sync.dma_start(out=at_sb, in_=at)
        at_pe = singles.tile([C, C], F32)  # A_T rows on partitions (PE rhs)
        nc.sync.dma_start(out=at_pe, in_=at[0, :, :])
        lens_sb = singles.tile([P, 1], F32)
        nc.sync.dma_start(out=lens_sb, in_=lens)
        ident = singles.tile([P, P], BF16)
        make_identity(nc, ident[:, :])

        iota_t = singles.tile([P, T], I32)
        nc.gpsimd.iota(iota_t, pattern=[[1, T]], base=0, channel_multiplier=0)
        iota_c = singles.tile([P, C], I32)
        nc.gpsimd.iota(iota_c, pattern=[[1, C]], base=0, channel_multiplier=0)
        iota_c_f = singles.tile([P, C], F32)
        nc.vector.tensor_copy(out=iota_c_f, in_=iota_c)

        # m[b,t] = t < len[b]; minv = !m. Integer masks for CopyPredicated
        # (and only plain tensor_copy may write u8 on this HW).
        mf = singles.tile([P, T], F32)
        nc.vector.tensor_scalar(out=mf, in0=iota_t, scalar1=lens_sb[:, :],
                                scalar2=None, op0=ALU.is_lt)
        minvf = singles.tile([P, T], F32)
        nc.vector.tensor_scalar(out=minvf, in0=mf, scalar1=-1.0, scalar2=1.0,
                                op0=ALU.mult, op1=ALU.add)
        m = singles.tile([P, T], U8)
        nc.vector.tensor_copy(out=m, in_=mf)
        minv = singles.tile([P, T], U8)
        nc.vector.tensor_copy(out=minv, in_=minvf)

        # score history: shist[:, t, :] = s_t
        shist = singles.tile([P, T, C], F32)
        nc.sync.dma_start(out=shist[:, 0, :], in_=pot[:, 0, :])

        # --- forward ---
        pot_sb = None
        for t in range(1, T):
            if t % CHUNK == 0 or pot_sb is None:
                c0 = (t // CHUNK) * CHUNK
                pot_sb = potp.tile([P, CHUNK, C], F32, tag="pot")
                nc.sync.dma_start(out=pot_sb, in_=pot[:, c0:c0 + CHUNK, :])
            col = t % CHUNK

            x = xp.tile([P, C, C], F32, tag="x")
            nc.vector._custom_dve(
                segop, out=x, in0=at_sb[:, :, :],
                in1=shist[:, t - 1, :].unsqueeze(1).broadcast_to([P, C, C]))
            nc.vector.tensor_tensor(shist[:, t, :], x[:, :, C - 1],
                                    pot_sb[:, col, :], ALU.add)
            nc.vector.copy_predicated(
                out=shist[:, t, :],
                mask=minv[:, t:t + 1].broadcast_to([P, C]),
                data=shist[:, t - 1, :])

        # --- last tag ---
        v8 = smal.tile([P, 8], F32, tag="v8")
        nc.vector.max(out=v8, in_=shist[:, T - 1, :])
        i8 = smal.tile([P, 8], U32, tag="i8")
        nc.vector.max_index(out=i8, in_max=v8, in_values=shist[:, T - 1, :])
        tag = singles.tile([P, 1], F32)
        nc.vector.tensor_copy(out=tag, in_=i8[:, 0:1])

        # --- backtrace ---
        # bf16 one-hot state: exact for 0/1, and the PE transpose of bf16 is
        # a single matmul pass (fp32 needs two).
        oh = smal.tile([P, C], BF16, tag="oh")
        nc.vector.tensor_scalar(out=oh, in0=iota_c_f, scalar1=tag[:, :],
                                scalar2=None, op0=ALU.is_equal)
        out_sb = outp.tile([P, CHUNK, C], F32, tag="oc")
        nc.scalar.copy(out=out_sb[:, CHUNK - 1, :], in_=oh)
        for t in range(T - 2, -1, -1):
            ci = t // CHUNK
            col = t % CHUNK
            if col == CHUNK - 1:
                new_sb = outp.tile([P, CHUNK, C], F32, tag="oc")
                out_sb_hi, out_sb = out_sb, new_sb
            ps_ohT = psp.tile([C, P], BF16, tag="ohT")
            nc.tensor.transpose(ps_ohT, oh, ident[:, :])
            sb_ohT = smal.tile([C, P], F32, tag="sbohT")
            nc.scalar.copy(out=sb_ohT, in_=ps_ohT)
            ps_asel = psp.tile([P, C], F32, tag="asel")
            nc.tensor.matmul(ps_asel, sb_ohT, at_pe, start=True, stop=True)
            xcol = smal.tile([P, C], F32, tag="xcol")
            nc.vector.tensor_tensor(xcol, shist[:, t, :], ps_asel, ALU.add)
            bv8 = smal.tile([P, 8], F32, tag="bv8")
            nc.vector.max(out=bv8, in_=xcol)
            bi8 = smal.tile([P, 8], U32, tag="bi8")
            nc.vector.max_index(out=bi8, in_max=bv8, in_values=xcol)
            tag_new = smal.tile([P, 1], F32, tag="tagn")
            nc.vector.tensor_copy(out=tag_new, in_=bi8[:, 0:1])
            nc.vector.copy_predicated(out=tag, mask=m[:, t + 1:t + 2],
                                      data=tag_new)
            oh = smal.tile([P, C], BF16, tag="oh")
            nc.vector.tensor_scalar(out=oh, in0=iota_c_f, scalar1=tag[:, :],
                                    scalar2=None, op0=ALU.is_equal)
            # output column (f32) off the critical chain, on the Scalar engine
            nc.scalar.copy(out=out_sb[:, col, :], in_=oh)
            if col == CHUNK - 1:
                hi0 = (ci + 1) * CHUNK
                nc.sync.dma_start(out=outd[:, hi0:hi0 + CHUNK, :],
                                  in_=out_sb_hi)
        nc.sync.dma_start(out=outd[:, 0:CHUNK, :], in_=out_sb)


def _build_module():
    import concourse.bacc as bacc
    import concourse.tile as tile
    from concourse import mybir

    F32 = mybir.dt.float32
    nc = bacc.Bacc("TRN2", debug=False, enable_asserts=False,
                   target_bir_lowering=False, num_devices=NCORES)
    ins = {
        "pot": nc.dram_tensor("pot", [P, T, C], F32, kind="ExternalInput").ap(),
        "lens": nc.dram_tensor("lens", [P, 1], F32, kind="ExternalInput").ap(),
        "at": nc.dram_tensor("at", [P, C, C], F32, kind="ExternalInput").ap(),
    }
    outs = {
        "out": nc.dram_tensor("out", [P, T, C], F32, kind="ExternalOutput").ap(),
    }
    with tile.TileContext(nc) as tc:
        crf_body(tc, outs, ins)
    nc.compile()
    return nc


def _get_module():
    if "nc" not in _CACHE:
        _CACHE["nc"] = _build_module()
    return _CACHE["nc"]


def _run(inputs, **spmd_kwargs):
    from concourse.bass_utils import run_bass_kernel_spmd

    potentials = np.ascontiguousarray(inputs["potentials"], dtype=np.float32)
    seq_lens = np.asarray(inputs["sequence_lengths"])
    transitions = np.ascontiguousarray(inputs["transitions"], dtype=np.float32)
    assert potentials.shape == (B, T, C)

    at_host = np.broadcast_to(
        np.ascontiguousarray(transitions.T)[None], (P, C, C))
    at_host = np.ascontiguousarray(at_host)
    lens_f = seq_lens.reshape(B, 1).astype(np.float32)

    in_maps = []
    for c in range(NCORES):
        sl = slice(c * P, (c + 1) * P)
        in_maps.append({
            "pot": np.ascontiguousarray(potentials[sl]),
            "lens": np.ascontiguousarray(lens_f[sl]),
            "at": at_host,
        })

    nc = _get_module()
    res = run_bass_kernel_spmd(nc, in_maps, core_ids=list(range(NCORES)),
                               **spmd_kwargs)
    out = np.concatenate([r["out"] for r in res.results], axis=0)
    return out.astype(np.float32), res


def kernel(**inputs) -> np.ndarray:
    out, _ = _run(inputs)
    return out


# revision 9
# speedup vs baseline: 1.2100x; 1.0193x over previous
"""CRF Viterbi decode (tf.contrib.crf.crf_decode + one_hot) on 8 TRN2 cores.

Data-parallel over batch: each of the 8 NeuronCores processes 128 of the
1024 sequences (batch rows on SBUF partitions). Per core the whole DP is
SBUF-resident and no per-step backpointers are materialized:

  forward (t = 1..511) keeps the full score history shist[:, t, :]:
    x[b,cc,cp] = A_T[cc,cp] + s_{t-1}[b,cp]     tensor_tensor add (s broadcast)
    raw[b,cc]  = max_cp x                       tensor_reduce axis=X
    s_t        = raw + pot[:,t]                 tensor_tensor add
    s_t        = s_{t-1} where t >= len[b]      copy_predicated

  backtrace recomputes only the traced argmax per step:
    ohT  = onehot(tag)^T (bf16)                 PE transpose
    asel = A[:, tag_b] per batch row            PE matmul (one-hot matvec, f32)
    xcol = s_t + asel                           tensor_tensor (PSUM src)
    tag  = first-argmax(xcol) if t+1 < len      Max8 + MaxIndex + copy_predicated
    out[:, t, :] = onehot(tag)                  tensor_scalar is_equal (+cast)

Score arithmetic replicates the reference's fp32 ops exactly (same adds,
exact max via one-hot matvec with exact 0/1 weights, first-index argmax),
so the output matches bit-for-bit.
"""
from contextlib import ExitStack

import numpy as np

B, T, C = 1024, 512, 48
NCORES = 8
P = B // NCORES  # 128 batch rows per core
CHUNK = 64

_CACHE = {}


def _register_seg_maxplus():
    """Custom DVE op: one streaming pass computing, per partition, the
    SEGMENTED running max of (in0 + in1) with segment length 48 (SUB_DIM
    boundaries of the 3D in0 AP). The segment tails are the grouped maxes.

    The stock Spec DSL has no segmented-scan reset, but the generated FSM
    already supports a SUB_DIM_DONE-triggered step state with per-stage
    overrides (production-tested by TENSOR_PAGED_MASK). We patch
    `_scan_overrides` to emit, for scans tagged `_ant_reset`, a step
    override that re-seeds the scan register from the current element:
    state := expr  (instead of state := max(state, expr)).
    """
    import concourse.dve_spec as ds
    from concourse.dve_ops import (OPS, CUSTOM_DVE_SPECS, DveOp,
                                   _SUB_OPCODE_FOR_NAME)
    from concourse.dve_uop import DveOpSpec

    name = "SEG_MAXPLUS_ANT"
    for o in OPS:
        if o.name == name:
            return o

    if not getattr(ds, "_ant_seg_patch", False):
        _orig_overrides = ds._scan_overrides

        def _patched(scans, node_stage):
            seed, step = _orig_overrides(scans, node_stage)
            for sc in scans:
                if getattr(sc, "_ant_reset", False):
                    step[node_stage[sc]] = ds._Stage(ds.AluOp.BYPASS, sc.expr)
            return seed, step

        ds._scan_overrides = _patched
        ds._ant_seg_patch = True

    body = ds.scan(ds.AluOp.MAX, ds.Src0 + ds.Src1)
    object.__setattr__(body, "_ant_reset", True)

    def _ref(in0, in1, c0, c1, c2):
        x = np.asarray(in0, np.float32) + np.asarray(in1, np.float32).reshape(
            in0.shape)
        p = x.shape[0]
        x3 = x.reshape(p, -1, x.shape[-1])
        return np.maximum.accumulate(x3, axis=-1).reshape(in0.shape)

    spec = ds.Spec(body=body, reference=_ref)
    row = 1 + len(OPS)
    _SUB_OPCODE_FOR_NAME[name] = row
    shas = {}
    for ver in ("v3", "v4"):
        try:
            shas[ver] = DveOpSpec(name=name, opcode=row,
                                  uops=ds.lower(spec, ver=ver),
                                  rd1_en=True).sha(ver)
        except Exception:
            pass
    op = DveOp(name, spec, subdim=True, uops_sha=shas)
    OPS.append(op)
    CUSTOM_DVE_SPECS[name] = spec
    return op


def crf_body(tc, outs, ins, T=T, CHUNK=CHUNK):
    import concourse.tile as tile  # noqa: F401
    from concourse import mybir
    from concourse.masks import make_identity

    F32 = mybir.dt.float32
    BF16 = mybir.dt.bfloat16
    U8 = mybir.dt.uint8
    I32 = mybir.dt.int32
    U32 = mybir.dt.uint32
    ALU = mybir.AluOpType

    nc = tc.nc
    segop = _register_seg_maxplus()
    pot = ins["pot"]      # [P, T, C] f32 dram
    lens = ins["lens"]    # [P, 1] f32 dram
    at = ins["at"]        # [P, C, C] f32 dram  (at[b,cc,cp] = A[cp,cc])
    outd = outs["out"]    # [P, T, C] f32 dram
    assert T % CHUNK == 0

    with ExitStack() as ctx:
        singles = ctx.enter_context(tc.tile_pool(name="singles", bufs=1))
        potp = ctx.enter_context(tc.tile_pool(name="potp", bufs=2))
        xp = ctx.enter_context(tc.tile_pool(name="xp", bufs=2))
        smal = ctx.enter_context(tc.tile_pool(name="smal", bufs=4))
        outp = ctx.enter_context(tc.tile_pool(name="outp", bufs=2))
        psp = ctx.enter_context(tc.tile_pool(name="psp", bufs=2, space="PSUM"))

        at_sb = singles.tile([P, C, C], F32)
        nc.sync.dma_start(out=at_sb, in_=at)
        at_pe = singles.tile([C, C], F32)  # A_T rows on partitions (PE rhs)
        nc.sync.dma_start(out=at_pe, in_=at[0, :, :])
        lens_sb = singles.tile([P, 1], F32)
        nc.sync.dma_start(out=lens_sb, in_=lens)
        ident = singles.tile([P, P], BF16)
        make_identity(nc, ident[:, :])

        iota_t = singles.tile([P, T], I32)
        nc.gpsimd.iota(iota_t, pattern=[[1, T]], base=0, channel_multiplier=0)
        iota_c = singles.tile([P, C], I32)
        nc.gpsimd.iota(iota_c, pattern=[[1, C]], base=0, channel_multiplier=0)
        iota_c_f = singles.tile([P, C], F32)
        nc.vector.tensor_copy(out=iota_c_f, in_=iota_c)

        # m[b,t] = t < len[b]; minv = !m. Integer masks for CopyPredicated
        # (and only plain tensor_copy may write u8 on this HW).
        mf = singles.tile([P, T], F32)
        nc.vector.tensor_scalar(out=mf, in0=iota_t, scalar1=lens_sb[:, :],
                                scalar2=None, op0=ALU.is_lt)
        m = singles.tile([P, T], U8)
        nc.vector.tensor_copy(out=m, in_=mf)

        # score history: shist[:, t, :] = s_t
        shist = singles.tile([P, T, C], F32)
        nc.sync.dma_start(out=shist[:, 0, :], in_=pot[:, 0, :])

        # --- forward ---
        pot_sb = None
        for t in range(1, T):
            if t % CHUNK == 0 or pot_sb is None:
                c0 = (t // CHUNK) * CHUNK
                pot_sb = potp.tile([P, CHUNK, C], F32, tag="pot")
                nc.sync.dma_start(out=pot_sb, in_=pot[:, c0:c0 + CHUNK, :])
            col = t % CHUNK

            x = xp.tile([P, C, C], F32, tag="x")
            nc.vector._custom_dve(
                segop, out=x, in0=at_sb[:, :, :],
                in1=shist[:, t - 1, :].unsqueeze(1).broadcast_to([P, C, C]))
            nc.vector.tensor_tensor(shist[:, t, :], x[:, :, C - 1],
                                    pot_sb[:, col, :], ALU.add)

        # --- last tag: argmax of s_{len[b]-1}, extracted from the history
        # via a one-hot-over-t masked sum (zeros elsewhere) ---
        lensm1 = singles.tile([P, 1], F32)
        nc.vector.tensor_scalar(out=lensm1, in0=lens_sb, scalar1=-1.0,
                                scalar2=None, op0=ALU.add)
        ohlen = singles.tile([P, T], F32)
        nc.vector.tensor_scalar(out=ohlen, in0=iota_t, scalar1=lensm1[:, :],
                                scalar2=None, op0=ALU.is_equal)
        svec = singles.tile([P, C], F32)
        nc.vector.memset(svec, 0.0)
        for k in range(T // CHUNK):
            lo = k * CHUNK
            prod = xp.tile([P, C, CHUNK], F32, tag="prod")
            nc.vector.tensor_tensor(
                prod, shist[:, lo:lo + CHUNK, :].transpose([0, 2, 1]),
                ohlen[:, lo:lo + CHUNK].unsqueeze(1).broadcast_to(
                    [P, C, CHUNK]), ALU.mult)
            part = smal.tile([P, C], F32, tag="part")
            nc.vector.tensor_reduce(out=part, in_=prod,
                                    axis=mybir.AxisListType.X, op=ALU.add)
            nc.vector.tensor_tensor(svec, svec, part, ALU.add)
        v8 = smal.tile([P, 8], F32, tag="v8")
        nc.vector.max(out=v8, in_=svec)
        i8 = smal.tile([P, 8], U32, tag="i8")
        nc.vector.max_index(out=i8, in_max=v8, in_values=svec)
        tag = singles.tile([P, 1], F32)
        nc.vector.tensor_copy(out=tag, in_=i8[:, 0:1])

        # --- backtrace ---
        # bf16 one-hot state: exact for 0/1, and the PE transpose of bf16 is
        # a single matmul pass (fp32 needs two).
        oh = smal.tile([P, C], BF16, tag="oh")
        nc.vector.tensor_scalar(out=oh, in0=iota_c_f, scalar1=tag[:, :],
                                scalar2=None, op0=ALU.is_equal)
        out_sb = outp.tile([P, CHUNK, C], F32, tag="oc")
        nc.scalar.copy(out=out_sb[:, CHUNK - 1, :], in_=oh)
        for t in range(T - 2, -1, -1):
            ci = t // CHUNK
            col = t % CHUNK
            if col == CHUNK - 1:
                new_sb = outp.tile([P, CHUNK, C], F32, tag="oc")
                out_sb_hi, out_sb = out_sb, new_sb
            ps_ohT = psp.tile([C, P], BF16, tag="ohT")
            nc.tensor.transpose(ps_ohT, oh, ident[:, :])
            sb_ohT = smal.tile([C, P], F32, tag="sbohT")
            nc.scalar.copy(out=sb_ohT, in_=ps_ohT)
            ps_asel = psp.tile([P, C], F32, tag="asel")
            nc.tensor.matmul(ps_asel, sb_ohT, at_pe, start=True, stop=True)
            xcol = smal.tile([P, C], F32, tag="xcol")
            nc.vector.tensor_tensor(xcol, shist[:, t, :], ps_asel, ALU.add)
            bv8 = smal.tile([P, 8], F32, tag="bv8")
            nc.vector.max(out=bv8, in_=xcol)
            bi8 = smal.tile([P, 8], U32, tag="bi8")
            nc.vector.max_index(out=bi8, in_max=bv8, in_values=xcol)
            tag_new = smal.tile([P, 1], F32, tag="tagn")
            nc.vector.tensor_copy(out=tag_new, in_=bi8[:, 0:1])
            nc.vector.copy_predicated(out=tag, mask=m[:, t + 1:t + 2],
                                      data=tag_new)
            oh = smal.tile([P, C], BF16, tag="oh")
            nc.vector.tensor_scalar(out=oh, in0=iota_c_f, scalar1=tag[:, :],
                                    scalar2=None, op0=ALU.is_equal)
            # output column (f32) off the critical chain, on the Scalar engine
            nc.scalar.copy(out=out_sb[:, col, :], in_=oh)
            if col == CHUNK - 1:
                hi0 = (ci + 1) * CHUNK
                nc.sync.dma_start(out=outd[:, hi0:hi0 + CHUNK, :],
                                  in_=out_sb_hi)
        nc.sync.dma_start(out=outd[:, 0:CHUNK, :], in_=out_sb)


def _build_module():
    import concourse.bacc as bacc
    import concourse.tile as tile
    from concourse import mybir

    F32 = mybir.dt.float32
    nc = bacc.Bacc("TRN2", debug=False, enable_asserts=False,
                   target_bir_lowering=False, num_devices=NCORES)
    ins = {
        "pot": nc.dram_tensor("pot", [P, T, C], F32, kind="ExternalInput").ap(),
        "lens": nc.dram_tensor("lens", [P, 1], F32, kind="ExternalInput").ap(),
        "at": nc.dram_tensor("at", [P, C, C], F32, kind="ExternalInput").ap(),
    }
    outs = {
        "out": nc.dram_tensor("out", [P, T, C], F32, kind="ExternalOutput").ap(),
    }
    with tile.TileContext(nc) as tc:
        crf_body(tc, outs, ins)
    nc.compile()
    return nc


def _get_module():
    if "nc" not in _CACHE:
        _CACHE["nc"] = _build_module()
    return _CACHE["nc"]


def _run(inputs, **spmd_kwargs):
    from concourse.bass_utils import run_bass_kernel_spmd

    potentials = np.ascontiguousarray(inputs["potentials"], dtype=np.float32)
    seq_lens = np.asarray(inputs["sequence_lengths"])
    transitions = np.ascontiguousarray(inputs["transitions"], dtype=np.float32)
    assert potentials.shape == (B, T, C)

    at_host = np.broadcast_to(
        np.ascontiguousarray(transitions.T)[None], (P, C, C))
    at_host = np.ascontiguousarray(at_host)
    lens_f = seq_lens.reshape(B, 1).astype(np.float32)

    in_maps = []
    for c in range(NCORES):
        sl = slice(c * P, (c + 1) * P)
        in_maps.append({
            "pot": np.ascontiguousarray(potentials[sl]),
            "lens": np.ascontiguousarray(lens_f[sl]),
            "at": at_host,
        })

    nc = _get_module()
    res = run_bass_kernel_spmd(nc, in_maps, core_ids=list(range(NCORES)),
                               **spmd_kwargs)
    out = np.concatenate([r["out"] for r in res.results], axis=0)
    return out.astype(np.float32), res


def kernel(**inputs) -> np.ndarray:
    out, _ = _run(inputs)
    return out


# revision 10
# speedup vs baseline: 1.3280x; 1.0975x over previous
"""CRF Viterbi decode (tf.contrib.crf.crf_decode + one_hot) on 8 TRN2 cores.

Data-parallel over batch: each of the 8 NeuronCores processes 128 of the
1024 sequences (batch rows on SBUF partitions). Per core the whole DP is
SBUF-resident and no per-step backpointers are materialized:

  forward (t = 1..511) keeps the full score history shist[:, t, :]:
    x[b,cc,cp] = A_T[cc,cp] + s_{t-1}[b,cp]     tensor_tensor add (s broadcast)
    raw[b,cc]  = max_cp x                       tensor_reduce axis=X
    s_t        = raw + pot[:,t]                 tensor_tensor add
    s_t        = s_{t-1} where t >= len[b]      copy_predicated

  backtrace recomputes only the traced argmax per step:
    ohT  = onehot(tag)^T (bf16)                 PE transpose
    asel = A[:, tag_b] per batch row            PE matmul (one-hot matvec, f32)
    xcol = s_t + asel                           tensor_tensor (PSUM src)
    tag  = first-argmax(xcol) if t+1 < len      Max8 + MaxIndex + copy_predicated
    out[:, t, :] = onehot(tag)                  tensor_scalar is_equal (+cast)

Score arithmetic replicates the reference's fp32 ops exactly (same adds,
exact max via one-hot matvec with exact 0/1 weights, first-index argmax),
so the output matches bit-for-bit.
"""
from contextlib import ExitStack

import numpy as np

B, T, C = 1024, 512, 48
NCORES = 8
P = B // NCORES  # 128 batch rows per core
CHUNK = 64

_CACHE = {}


def _register_seg_maxplus():
    """Custom DVE op: one streaming pass computing, per partition, the
    SEGMENTED running max of (in0 + in1) with segment length 48 (SUB_DIM
    boundaries of the 3D in0 AP). The segment tails are the grouped maxes.

    The stock Spec DSL has no segmented-scan reset, but the generated FSM
    already supports a SUB_DIM_DONE-triggered step state with per-stage
    overrides (production-tested by TENSOR_PAGED_MASK). We patch
    `_scan_overrides` to emit, for scans tagged `_ant_reset`, a step
    override that re-seeds the scan register from the current element:
    state := expr  (instead of state := max(state, expr)).
    """
    import concourse.dve_spec as ds
    from concourse.dve_ops import (OPS, CUSTOM_DVE_SPECS, DveOp,
                                   _SUB_OPCODE_FOR_NAME)
    from concourse.dve_uop import DveOpSpec

    name = "SEG_MAXPLUS_ANT"
    for o in OPS:
        if o.name == name:
            return o

    if not getattr(ds, "_ant_seg_patch", False):
        _orig_overrides = ds._scan_overrides

        def _patched(scans, node_stage):
            seed, step = _orig_overrides(scans, node_stage)
            for sc in scans:
                if getattr(sc, "_ant_reset", False):
                    step[node_stage[sc]] = ds._Stage(ds.AluOp.BYPASS, sc.expr)
            return seed, step

        ds._scan_overrides = _patched
        ds._ant_seg_patch = True

    body = ds.scan(ds.AluOp.MAX, ds.Src0 + ds.Src1)
    object.__setattr__(body, "_ant_reset", True)

    def _ref(in0, in1, c0, c1, c2):
        x = np.asarray(in0, np.float32) + np.asarray(in1, np.float32).reshape(
            in0.shape)
        p = x.shape[0]
        x3 = x.reshape(p, -1, x.shape[-1])
        return np.maximum.accumulate(x3, axis=-1).reshape(in0.shape)

    spec = ds.Spec(body=body, reference=_ref)
    row = 1 + len(OPS)
    _SUB_OPCODE_FOR_NAME[name] = row
    shas = {}
    for ver in ("v3", "v4"):
        try:
            shas[ver] = DveOpSpec(name=name, opcode=row,
                                  uops=ds.lower(spec, ver=ver),
                                  rd1_en=True).sha(ver)
        except Exception:
            pass
    op = DveOp(name, spec, subdim=True, uops_sha=shas)
    OPS.append(op)
    CUSTOM_DVE_SPECS[name] = spec
    return op


def crf_body(tc, outs, ins, T=T, CHUNK=CHUNK):
    import concourse.tile as tile  # noqa: F401
    from concourse import mybir
    from concourse.masks import make_identity

    F32 = mybir.dt.float32
    BF16 = mybir.dt.bfloat16
    U8 = mybir.dt.uint8
    I32 = mybir.dt.int32
    U32 = mybir.dt.uint32
    ALU = mybir.AluOpType

    nc = tc.nc
    segop = _register_seg_maxplus()
    pot = ins["pot"]      # [P, T, C] f32 dram
    lens = ins["lens"]    # [P, 1] f32 dram
    at = ins["at"]        # [P, C, C] f32 dram  (at[b,cc,cc] = A[cp,cc])
    atsplit = ins["atsplit"]  # [3, C, C] bf16 dram
    outd = outs["out"]    # [P, T, C] f32 dram
    assert T % CHUNK == 0

    with ExitStack() as ctx:
        singles = ctx.enter_context(tc.tile_pool(name="singles", bufs=1))
        potp = ctx.enter_context(tc.tile_pool(name="potp", bufs=2))
        xp = ctx.enter_context(tc.tile_pool(name="xp", bufs=2))
        smal = ctx.enter_context(tc.tile_pool(name="smal", bufs=4))
        outp = ctx.enter_context(tc.tile_pool(name="outp", bufs=2))
        psp = ctx.enter_context(tc.tile_pool(name="psp", bufs=2, space="PSUM"))

        at_sb = singles.tile([P, C, C], F32)
        nc.sync.dma_start(out=at_sb, in_=at)
        at3 = singles.tile([C, 3, C], BF16)  # A_T = h1+h2+h3 exact bf16 split
        nc.sync.dma_start(out=at3, in_=atsplit.transpose([1, 0, 2]))
        lens_sb = singles.tile([P, 1], F32)
        nc.sync.dma_start(out=lens_sb, in_=lens)
        ident = singles.tile([P, P], BF16)
        make_identity(nc, ident[:, :])

        iota_t = singles.tile([P, T], I32)
        nc.gpsimd.iota(iota_t, pattern=[[1, T]], base=0, channel_multiplier=0)
        iota_c = singles.tile([P, C], I32)
        nc.gpsimd.iota(iota_c, pattern=[[1, C]], base=0, channel_multiplier=0)
        iota_c_f = singles.tile([P, C], F32)
        nc.vector.tensor_copy(out=iota_c_f, in_=iota_c)

        # m[b,t] = t < len[b]; minv = !m. Integer masks for CopyPredicated
        # (and only plain tensor_copy may write u8 on this HW).
        mf = singles.tile([P, T], F32)
        nc.vector.tensor_scalar(out=mf, in0=iota_t, scalar1=lens_sb[:, :],
                                scalar2=None, op0=ALU.is_lt)
        m = singles.tile([P, T], U8)
        nc.vector.tensor_copy(out=m, in_=mf)

        # score history: shist[:, t, :] = s_t
        shist = singles.tile([P, T, C], F32)
        nc.sync.dma_start(out=shist[:, 0, :], in_=pot[:, 0, :])

        # --- forward ---
        pot_sb = None
        for t in range(1, T):
            if t % CHUNK == 0 or pot_sb is None:
                c0 = (t // CHUNK) * CHUNK
                pot_sb = potp.tile([P, CHUNK, C], F32, tag="pot")
                nc.sync.dma_start(out=pot_sb, in_=pot[:, c0:c0 + CHUNK, :])
            col = t % CHUNK

            x = xp.tile([P, C, C], F32, tag="x")
            nc.vector._custom_dve(
                segop, out=x, in0=at_sb[:, :, :],
                in1=shist[:, t - 1, :].unsqueeze(1).broadcast_to([P, C, C]))
            nc.vector.tensor_tensor(shist[:, t, :], x[:, :, C - 1],
                                    pot_sb[:, col, :], ALU.add)

        # --- last tag: argmax of s_{len[b]-1}, extracted from the history
        # via a one-hot-over-t masked sum (zeros elsewhere) ---
        lensm1 = singles.tile([P, 1], F32)
        nc.vector.tensor_scalar(out=lensm1, in0=lens_sb, scalar1=-1.0,
                                scalar2=None, op0=ALU.add)
        ohlen = singles.tile([P, T], F32)
        nc.vector.tensor_scalar(out=ohlen, in0=iota_t, scalar1=lensm1[:, :],
                                scalar2=None, op0=ALU.is_equal)
        svec = singles.tile([P, C], F32)
        nc.vector.memset(svec, 0.0)
        for k in range(T // CHUNK):
            lo = k * CHUNK
            prod = xp.tile([P, C, CHUNK], F32, tag="prod")
            nc.vector.tensor_tensor(
                prod, shist[:, lo:lo + CHUNK, :].transpose([0, 2, 1]),
                ohlen[:, lo:lo + CHUNK].unsqueeze(1).broadcast_to(
                    [P, C, CHUNK]), ALU.mult)
            part = smal.tile([P, C], F32, tag="part")
            nc.vector.tensor_reduce(out=part, in_=prod,
                                    axis=mybir.AxisListType.X, op=ALU.add)
            nc.vector.tensor_tensor(svec, svec, part, ALU.add)
        v8 = smal.tile([P, 8], F32, tag="v8")
        nc.vector.max(out=v8, in_=svec)
        i8 = smal.tile([P, 8], U32, tag="i8")
        nc.vector.max_index(out=i8, in_max=v8, in_values=svec)
        tag = singles.tile([P, 1], F32)
        nc.vector.tensor_copy(out=tag, in_=i8[:, 0:1])

        # --- backtrace ---
        # bf16 one-hot state: exact for 0/1, and the PE transpose of bf16 is
        # a single matmul pass (fp32 needs two).
        oh = smal.tile([P, C], BF16, tag="oh")
        nc.vector.tensor_scalar(out=oh, in0=iota_c_f, scalar1=tag[:, :],
                                scalar2=None, op0=ALU.is_equal)
        out_sb = outp.tile([P, CHUNK, C], F32, tag="oc")
        nc.scalar.copy(out=out_sb[:, CHUNK - 1, :], in_=oh)
        for t in range(T - 2, -1, -1):
            ci = t // CHUNK
            col = t % CHUNK
            if col == CHUNK - 1:
                new_sb = outp.tile([P, CHUNK, C], F32, tag="oc")
                out_sb_hi, out_sb = out_sb, new_sb
            ps_ohT = psp.tile([C, P], BF16, tag="ohT")
            nc.tensor.transpose(ps_ohT, oh, ident[:, :])
            sb_ohT = smal.tile([C, P], BF16, tag="sbohT")
            nc.vector.tensor_copy(out=sb_ohT, in_=ps_ohT)
            ps_asel = psp.tile([P, C], F32, tag="asel")
            nc.tensor.matmul(ps_asel, sb_ohT, at3[:, 0, :],
                             start=True, stop=False)
            nc.tensor.matmul(ps_asel, sb_ohT, at3[:, 1, :],
                             start=False, stop=False)
            nc.tensor.matmul(ps_asel, sb_ohT, at3[:, 2, :],
                             start=False, stop=True)
            xcol = smal.tile([P, C], F32, tag="xcol")
            nc.vector.tensor_tensor(xcol, shist[:, t, :], ps_asel, ALU.add)
            bv8 = smal.tile([P, 8], F32, tag="bv8")
            nc.vector.max(out=bv8, in_=xcol)
            bi8 = smal.tile([P, 8], U32, tag="bi8")
            nc.vector.max_index(out=bi8, in_max=bv8, in_values=xcol)
            tag_new = smal.tile([P, 1], F32, tag="tagn")
            nc.vector.tensor_copy(out=tag_new, in_=bi8[:, 0:1])
            nc.vector.copy_predicated(out=tag, mask=m[:, t + 1:t + 2],
                                      data=tag_new)
            oh = smal.tile([P, C], BF16, tag="oh")
            nc.vector.tensor_scalar(out=oh, in0=iota_c_f, scalar1=tag[:, :],
                                    scalar2=None, op0=ALU.is_equal)
            # output column (f32) off the critical chain, on the Scalar engine
            nc.scalar.copy(out=out_sb[:, col, :], in_=oh)
            if col == CHUNK - 1:
                hi0 = (ci + 1) * CHUNK
                nc.sync.dma_start(out=outd[:, hi0:hi0 + CHUNK, :],
                                  in_=out_sb_hi)
        nc.sync.dma_start(out=outd[:, 0:CHUNK, :], in_=out_sb)


def _build_module():
    import concourse.bacc as bacc
    import concourse.tile as tile
    from concourse import mybir

    F32 = mybir.dt.float32
    nc = bacc.Bacc("TRN2", debug=False, enable_asserts=False,
                   target_bir_lowering=False, num_devices=NCORES)
    ins = {
        "pot": nc.dram_tensor("pot", [P, T, C], F32, kind="ExternalInput").ap(),
        "lens": nc.dram_tensor("lens", [P, 1], F32, kind="ExternalInput").ap(),
        "at": nc.dram_tensor("at", [P, C, C], F32, kind="ExternalInput").ap(),
        "atsplit": nc.dram_tensor("atsplit", [3, C, C], mybir.dt.bfloat16,
                                  kind="ExternalInput").ap(),
    }
    outs = {
        "out": nc.dram_tensor("out", [P, T, C], F32, kind="ExternalOutput").ap(),
    }
    with tile.TileContext(nc) as tc:
        crf_body(tc, outs, ins)
    nc.compile()
    return nc


def _get_module():
    if "nc" not in _CACHE:
        _CACHE["nc"] = _build_module()
    return _CACHE["nc"]


def _run(inputs, **spmd_kwargs):
    from concourse.bass_utils import run_bass_kernel_spmd

    potentials = np.ascontiguousarray(inputs["potentials"], dtype=np.float32)
    seq_lens = np.asarray(inputs["sequence_lengths"])
    transitions = np.ascontiguousarray(inputs["transitions"], dtype=np.float32)
    assert potentials.shape == (B, T, C)

    at_host = np.broadcast_to(
        np.ascontiguousarray(transitions.T)[None], (P, C, C))
    at_host = np.ascontiguousarray(at_host)
    import ml_dtypes
    a0 = np.ascontiguousarray(transitions.T).astype(np.float32)
    h1 = a0.astype(ml_dtypes.bfloat16)
    r1 = a0 - h1.astype(np.float32)
    h2 = r1.astype(ml_dtypes.bfloat16)
    h3 = (r1 - h2.astype(np.float32)).astype(ml_dtypes.bfloat16)
    assert (h1.astype(np.float32) + h2.astype(np.float32)
            + h3.astype(np.float32) == a0).all(), "bf16 split not exact"
    atsplit = np.ascontiguousarray(np.stack([h1, h2, h3]))
    lens_f = seq_lens.reshape(B, 1).astype(np.float32)

    in_maps = []
    for c in range(NCORES):
        sl = slice(c * P, (c + 1) * P)
        in_maps.append({
            "pot": np.ascontiguousarray(potentials[sl]),
            "lens": np.ascontiguousarray(lens_f[sl]),
            "at": at_host,
            "atsplit": atsplit,
        })

    nc = _get_module()
    res = run_bass_kernel_spmd(nc, in_maps, core_ids=list(range(NCORES)),
                               **spmd_kwargs)
    out = np.concatenate([r["out"] for r in res.results], axis=0)
    return out.astype(np.float32), res


def kernel(**inputs) -> np.ndarray:
    out, _ = _run(inputs)
    return out


# revision 13
# speedup vs baseline: 1.3294x; 1.0011x over previous
"""CRF Viterbi decode (tf.contrib.crf.crf_decode + one_hot) on 8 TRN2 cores.

Data-parallel over batch: each of the 8 NeuronCores processes 128 of the
1024 sequences (batch rows on SBUF partitions). Per core the whole DP is
SBUF-resident and no per-step backpointers are materialized:

  forward (t = 1..511) keeps the full score history shist[:, t, :]:
    x[b,cc,cp] = A_T[cc,cp] + s_{t-1}[b,cp]     tensor_tensor add (s broadcast)
    raw[b,cc]  = max_cp x                       tensor_reduce axis=X
    s_t        = raw + pot[:,t]                 tensor_tensor add
    s_t        = s_{t-1} where t >= len[b]      copy_predicated

  backtrace recomputes only the traced argmax per step:
    ohT  = onehot(tag)^T (bf16)                 PE transpose
    asel = A[:, tag_b] per batch row            PE matmul (one-hot matvec, f32)
    xcol = s_t + asel                           tensor_tensor (PSUM src)
    tag  = first-argmax(xcol) if t+1 < len      Max8 + MaxIndex + copy_predicated
    out[:, t, :] = onehot(tag)                  tensor_scalar is_equal (+cast)

Score arithmetic replicates the reference's fp32 ops exactly (same adds,
exact max via one-hot matvec with exact 0/1 weights, first-index argmax),
so the output matches bit-for-bit.
"""
from contextlib import ExitStack

import numpy as np

B, T, C = 1024, 512, 48
NCORES = 8
P = B // NCORES  # 128 batch rows per core
CHUNK = 64

_CACHE = {}


def _register_seg_maxplus():
    """Custom DVE op: one streaming pass computing, per partition, the
    SEGMENTED running max of (in0 + in1) with segment length 48 (SUB_DIM
    boundaries of the 3D in0 AP). The segment tails are the grouped maxes.

    The stock Spec DSL has no segmented-scan reset, but the generated FSM
    already supports a SUB_DIM_DONE-triggered step state with per-stage
    overrides (production-tested by TENSOR_PAGED_MASK). We patch
    `_scan_overrides` to emit, for scans tagged `_ant_reset`, a step
    override that re-seeds the scan register from the current element:
    state := expr  (instead of state := max(state, expr)).
    """
    import concourse.dve_spec as ds
    from concourse.dve_ops import (OPS, CUSTOM_DVE_SPECS, DveOp,
                                   _SUB_OPCODE_FOR_NAME)
    from concourse.dve_uop import DveOpSpec

    name = "SEG_MAXPLUS_ANT"
    for o in OPS:
        if o.name == name:
            return o

    if not getattr(ds, "_ant_seg_patch", False):
        _orig_overrides = ds._scan_overrides

        def _patched(scans, node_stage):
            seed, step = _orig_overrides(scans, node_stage)
            for sc in scans:
                if getattr(sc, "_ant_reset", False):
                    step[node_stage[sc]] = ds._Stage(ds.AluOp.BYPASS, sc.expr)
            return seed, step

        ds._scan_overrides = _patched
        ds._ant_seg_patch = True

    body = ds.scan(ds.AluOp.MAX, ds.Src0 + ds.Src1)
    object.__setattr__(body, "_ant_reset", True)

    def _ref(in0, in1, c0, c1, c2):
        x = np.asarray(in0, np.float32) + np.asarray(in1, np.float32).reshape(
            in0.shape)
        p = x.shape[0]
        x3 = x.reshape(p, -1, x.shape[-1])
        return np.maximum.accumulate(x3, axis=-1).reshape(in0.shape)

    spec = ds.Spec(body=body, reference=_ref)
    row = 1 + len(OPS)
    _SUB_OPCODE_FOR_NAME[name] = row
    shas = {}
    for ver in ("v3", "v4"):
        try:
            shas[ver] = DveOpSpec(name=name, opcode=row,
                                  uops=ds.lower(spec, ver=ver),
                                  rd1_en=True).sha(ver)
        except Exception:
            pass
    op = DveOp(name, spec, subdim=True, uops_sha=shas)
    OPS.append(op)
    CUSTOM_DVE_SPECS[name] = spec
    return op


def crf_body(tc, outs, ins, T=T, CHUNK=CHUNK):
    import concourse.tile as tile  # noqa: F401
    from concourse import mybir
    from concourse.masks import make_identity

    F32 = mybir.dt.float32
    BF16 = mybir.dt.bfloat16
    U8 = mybir.dt.uint8
    I32 = mybir.dt.int32
    U32 = mybir.dt.uint32
    ALU = mybir.AluOpType

    nc = tc.nc
    segop = _register_seg_maxplus()
    pot = ins["pot"]      # [P, T, C] f32 dram
    lens = ins["lens"]    # [P, 1] f32 dram
    at = ins["at"]        # [P, C, C] f32 dram  (at[b,cc,cc] = A[cp,cc])
    atsplit = ins["atsplit"]  # [3, C, C] bf16 dram
    outd = outs["out"]    # [P, T, C] f32 dram
    assert T % CHUNK == 0

    with ExitStack() as ctx:
        singles = ctx.enter_context(tc.tile_pool(name="singles", bufs=1))
        potp = ctx.enter_context(tc.tile_pool(name="potp", bufs=2))
        xp = ctx.enter_context(tc.tile_pool(name="xp", bufs=2))
        smal = ctx.enter_context(tc.tile_pool(name="smal", bufs=4))
        outp = ctx.enter_context(tc.tile_pool(name="outp", bufs=2))
        psp = ctx.enter_context(tc.tile_pool(name="psp", bufs=2, space="PSUM"))

        at_sb = singles.tile([P, C, C], F32)
        nc.sync.dma_start(out=at_sb, in_=at)
        at3 = singles.tile([C, 3, C], BF16)  # A_T = h1+h2+h3 exact bf16 split
        nc.sync.dma_start(out=at3, in_=atsplit.transpose([1, 0, 2]))
        lens_sb = singles.tile([P, 1], F32)
        nc.sync.dma_start(out=lens_sb, in_=lens)
        ident = singles.tile([P, P], BF16)
        make_identity(nc, ident[:, :])

        iota_t = singles.tile([P, T], I32)
        nc.gpsimd.iota(iota_t, pattern=[[1, T]], base=0, channel_multiplier=0)
        iota_c = singles.tile([P, C], I32)
        nc.gpsimd.iota(iota_c, pattern=[[1, C]], base=0, channel_multiplier=0)
        iota_c_f = singles.tile([P, C], F32)
        nc.vector.tensor_copy(out=iota_c_f, in_=iota_c)

        # m[b,t] = t < len[b]; minv = !m. Integer masks for CopyPredicated
        # (and only plain tensor_copy may write u8 on this HW).
        mf = singles.tile([P, T], F32)
        nc.vector.tensor_scalar(out=mf, in0=iota_t, scalar1=lens_sb[:, :],
                                scalar2=None, op0=ALU.is_lt)
        m = singles.tile([P, T], U8)
        nc.vector.tensor_copy(out=m, in_=mf)

        # score history: shist[:, t, :] = s_t
        shist = singles.tile([P, T, C], F32)
        nc.sync.dma_start(out=shist[:, 0, :], in_=pot[:, 0, :])

        # --- forward ---
        pot_sb = None
        for t in range(1, T):
            if t % CHUNK == 0 or pot_sb is None:
                c0 = (t // CHUNK) * CHUNK
                pot_sb = potp.tile([P, CHUNK, C], F32, tag="pot")
                nc.sync.dma_start(out=pot_sb, in_=pot[:, c0:c0 + CHUNK, :])
            col = t % CHUNK

            x = xp.tile([P, C, C], F32, tag="x")
            nc.vector._custom_dve(
                segop, out=x, in0=at_sb[:, :, :],
                in1=shist[:, t - 1, :].unsqueeze(1).broadcast_to([P, C, C]))
            nc.vector.tensor_tensor(shist[:, t, :], x[:, :, C - 1],
                                    pot_sb[:, col, :], ALU.add)

        # --- last tag: argmax of s_{len[b]-1}, extracted from the history
        # via a one-hot-over-t masked sum (zeros elsewhere) ---
        lensm1 = singles.tile([P, 1], F32)
        nc.vector.tensor_scalar(out=lensm1, in0=lens_sb, scalar1=-1.0,
                                scalar2=None, op0=ALU.add)
        ohlen = singles.tile([P, T], F32)
        nc.vector.tensor_scalar(out=ohlen, in0=iota_t, scalar1=lensm1[:, :],
                                scalar2=None, op0=ALU.is_equal)
        svec = singles.tile([P, C], F32)
        nc.vector.memset(svec, 0.0)
        for k in range(T // CHUNK):
            lo = k * CHUNK
            prod = xp.tile([P, C, CHUNK], F32, tag="prod")
            nc.vector.tensor_tensor(
                prod, shist[:, lo:lo + CHUNK, :].transpose([0, 2, 1]),
                ohlen[:, lo:lo + CHUNK].unsqueeze(1).broadcast_to(
                    [P, C, CHUNK]), ALU.mult)
            part = smal.tile([P, C], F32, tag="part")
            nc.vector.tensor_reduce(out=part, in_=prod,
                                    axis=mybir.AxisListType.X, op=ALU.add)
            nc.vector.tensor_tensor(svec, svec, part, ALU.add)
        v8 = smal.tile([P, 8], F32, tag="v8")
        nc.vector.max(out=v8, in_=svec)
        i8 = smal.tile([P, 8], U32, tag="i8")
        nc.vector.max_index(out=i8, in_max=v8, in_values=svec)
        tag = singles.tile([P, 1], F32)
        nc.vector.tensor_copy(out=tag, in_=i8[:, 0:1])

        # --- backtrace ---
        # bf16 one-hot state: exact for 0/1, and the PE transpose of bf16 is
        # a single matmul pass (fp32 needs two).
        oh = smal.tile([P, C], BF16, tag="oh")
        nc.vector.tensor_scalar(out=oh, in0=iota_c_f, scalar1=tag[:, :],
                                scalar2=None, op0=ALU.is_equal)
        out_sb = outp.tile([P, CHUNK, C], F32, tag="oc")
        nc.scalar.copy(out=out_sb[:, CHUNK - 1, :], in_=oh)
        for t in range(T - 2, -1, -1):
            ci = t // CHUNK
            col = t % CHUNK
            if col == CHUNK - 1:
                new_sb = outp.tile([P, CHUNK, C], F32, tag="oc")
                out_sb_hi, out_sb = out_sb, new_sb
            ps_ohT = psp.tile([C, P], BF16, tag="ohT")
            nc.tensor.transpose(ps_ohT, oh, ident[:, :])
            sb_ohT = smal.tile([C, P], BF16, tag="sbohT")
            nc.vector.tensor_copy(out=sb_ohT, in_=ps_ohT)
            ps_asel = psp.tile([P, C], F32, tag="asel")
            nc.tensor.matmul(ps_asel, sb_ohT, at3[:, 0, :],
                             start=True, stop=False)
            nc.tensor.matmul(ps_asel, sb_ohT, at3[:, 1, :],
                             start=False, stop=False)
            nc.tensor.matmul(ps_asel, sb_ohT, at3[:, 2, :],
                             start=False, stop=True)
            xcol = smal.tile([P, C], F32, tag="xcol")
            nc.vector.tensor_tensor(xcol, shist[:, t, :], ps_asel, ALU.add)
            bv8 = smal.tile([P, 8], F32, tag="bv8")
            nc.vector.max(out=bv8, in_=xcol)
            bi8 = smal.tile([P, 8], U32, tag="bi8")
            nc.vector.max_index(out=bi8, in_max=bv8, in_values=xcol)
            tag_new = smal.tile([P, 1], F32, tag="tagn")
            nc.vector.tensor_copy(out=tag_new, in_=bi8[:, 0:1])
            nc.vector.copy_predicated(out=tag, mask=m[:, t + 1:t + 2],
                                      data=tag_new)
            oh = smal.tile([P, C], BF16, tag="oh")
            nc.vector.tensor_scalar(out=oh, in0=iota_c_f, scalar1=tag[:, :],
                                    scalar2=None, op0=ALU.is_equal)
            # output column (f32) off the critical chain, on the Scalar engine
            nc.scalar.copy(out=out_sb[:, col, :], in_=oh)
            if col == CHUNK - 1:
                hi0 = (ci + 1) * CHUNK
                nc.sync.dma_start(out=outd[:, hi0:hi0 + CHUNK, :],
                                  in_=out_sb_hi)
        nc.sync.dma_start(out=outd[:, 0:CHUNK, :], in_=out_sb)


def _build_module():
    import concourse.bacc as bacc
    import concourse.tile as tile
    from concourse import mybir

    F32 = mybir.dt.float32
    nc = bacc.Bacc("TRN2", debug=False, enable_asserts=False,
                   target_bir_lowering=False, num_devices=NCORES)
    ins = {
        "pot": nc.dram_tensor("pot", [P, T, C], F32, kind="ExternalInput").ap(),
        "lens": nc.dram_tensor("lens", [P, 1], F32, kind="ExternalInput").ap(),
        "at": nc.dram_tensor("at", [P, C, C], F32, kind="ExternalInput").ap(),
        "atsplit": nc.dram_tensor("atsplit", [3, C, C], mybir.dt.bfloat16,
                                  kind="ExternalInput").ap(),
    }
    outs = {
        "out": nc.dram_tensor("out", [P, T, C], F32, kind="ExternalOutput").ap(),
    }
    with tile.TileContext(nc) as tc:
        crf_body(tc, outs, ins)
    nc.compile()
    return nc


def _get_module():
    if "nc" not in _CACHE:
        _CACHE["nc"] = _build_module()
    return _CACHE["nc"]


def _run(inputs, **spmd_kwargs):
    from concourse.bass_utils import run_bass_kernel_spmd

    potentials = np.ascontiguousarray(inputs["potentials"], dtype=np.float32)
    seq_lens = np.asarray(inputs["sequence_lengths"])
    transitions = np.ascontiguousarray(inputs["transitions"], dtype=np.float32)
    assert potentials.shape == (B, T, C)

    at_host = np.broadcast_to(
        np.ascontiguousarray(transitions.T)[None], (P, C, C))
    at_host = np.ascontiguousarray(at_host)
    import ml_dtypes
    a0 = np.ascontiguousarray(transitions.T).astype(np.float32)
    h1 = a0.astype(ml_dtypes.bfloat16)
    r1 = a0 - h1.astype(np.float32)
    h2 = r1.astype(ml_dtypes.bfloat16)
    h3 = (r1 - h2.astype(np.float32)).astype(ml_dtypes.bfloat16)
    assert (h1.astype(np.float32) + h2.astype(np.float32)
            + h3.astype(np.float32) == a0).all(), "bf16 split not exact"
    atsplit = np.ascontiguousarray(np.stack([h1, h2, h3]))
    lens_f = seq_lens.reshape(B, 1).astype(np.float32)

    in_maps = []
    for c in range(NCORES):
        sl = slice(c * P, (c + 1) * P)
        in_maps.append({
            "pot": np.ascontiguousarray(potentials[sl]),
            "lens": np.ascontiguousarray(lens_f[sl]),
            "at": at_host,
            "atsplit": atsplit,
        })

    nc = _get_module()
    res = run_bass_kernel_spmd(nc, in_maps, core_ids=list(range(NCORES)),
                               **spmd_kwargs)
    out = np.concatenate([r["out"] for r in res.results], axis=0)
    return out.astype(np.float32), res


def kernel(**inputs) -> np.ndarray:
    out, _ = _run(inputs)
    return out
